# revision 7
# baseline (speedup 1.0000x reference)
"""LSTM decoder with attention (image captioning) — Trainium2 Bass kernel.

Sharding: data-parallel over batch (64 images -> 8 cores x 8 images).
Host does cheap glue (embedding gather, weight transposes, h0/c0 init,
final bias add).

v2 design (vs the IFW-precompute baseline):
  - two-stage context each step (ctx = IF.T @ w, then gates += Wc @ ctx)
    instead of precomputing IFW = IF @ Wc.T (saves ~13 GFLOP of PE time)
  - all-tanh LSTM cell: sigmoid(x) = (1+tanh(x/2))/2 with h,c carried as
    2h, 2c and the consuming weights pre-halved on the host -> every
    activation (att-tanh, exp, cell) lives in one ACT table set, zero
    ACT_TABLE_LOADs in steady state
  - attention bias-add done on DVE (tensor_scalar per image), tanh as 4
    whole-row [128, 1568] ACT instructions
  - W_hh part of the gates issued at step start (before the attention
    serial chain) to keep the PE warm
  - tight P=196 layouts everywhere (no 256 padding)

Device program per core (b = 8 local images):
  pre:   enc_projT[a, (b,q)] = wenc @ IF.T
  loop (t = 0..19):
         hprojT = (wdec/2) @ H2          (H2 = 2h)
         arg = encp + hproj (DVE), att = tanh(arg) (ACT)
         eT[q,b] = V . att (PE), softmax via exp + ones-matmul sums
         gates = Whh-part (early) + Wc @ (IF.T @ w) + embproj[t]
         cell in tanh form -> H2[t+1], W2 (= 2c)
  tail:  logits = H2.T @ (fc/2).T   (streamed fc chunks)
"""

import os
import sys
import numpy as np

for _p in ("/opt/trn_rl_repo",):
    if _p not in sys.path and os.path.isdir(_p):
        sys.path.insert(0, _p)

import ml_dtypes  # noqa: E402

import concourse.bass as bass  # noqa: E402
import concourse.tile as tile  # noqa: E402
from concourse import bacc, mybir  # noqa: E402
from concourse.bass_utils import run_bass_kernel_spmd  # noqa: E402

AF = mybir.ActivationFunctionType
ALU = mybir.AluOpType
F32 = mybir.dt.float32
BF16 = mybir.dt.bfloat16
BF = ml_dtypes.bfloat16

# problem shapes (hardcoded)
VOCAB, ENC, EMB, DEC, ATT = 10000, 2048, 512, 512, 512
B, P, S = 64, 196, 20
NCORES = 8
NB = B // NCORES          # 8 images per core
PB = NB * P               # 1568 packed (b,q) columns
J1 = P - 128              # 68 rows in the second q-tile of each image
NE = ENC // 128           # 16
NA = ATT // 128           # 4
ND = DEC // 128           # 4
NMT = 16                  # gate m-tiles (gate-major: mt = gate*4 + r)
NK2 = NE + ND             # 20 stage-2 contraction tiles (ctx then h)
D4 = 4 * DEC              # 2048
VC = 500                  # vocab chunk
NVC = VOCAB // VC         # 20
EC = PB // 4              # 392 encp pre-chunk

_CACHE = {}
TRACE = False  # set by test.py to capture an NTFF profile


def _build_nc():
    if "nc" in _CACHE:
        return _CACHE["nc"]

    nc = bacc.Bacc(
        "TRN2",
        target_bir_lowering=False,
        debug=False,
        enable_asserts=False,
        num_devices=NCORES,
    )

    def din(name, shape, dt=BF16):
        return nc.dram_tensor(name, shape, dt, kind="ExternalInput").ap()

    ift_d = din("ift", [NE, 128, PB])              # IF.T  [e, (b,q)]
    ifp_d = din("ifp", [2 * NB, 128, D4])          # IF    [(b,j) tiles: q, e]
    wenct_d = din("wenct", [NE, 128, ATT])         # wenc.T [e, a]
    w2ct_d = din("w2ct", [NK2, 128, D4])           # [Wc.T ; (W_hh/2).T]
    wdect_d = din("wdect", [ND, 128, ATT])         # (wdec/2).T [dec, a]
    vt_d = din("vt", [NA, 128, 1])                 # V_w.T
    encb_d = din("encb", [NA, 128, 1], F32)
    wdecb_d = din("wdecb", [NA, 128, 1], F32)
    ept_d = din("ept", [128, S * NMT * NB])        # embproj [r, (t, mt, b)]
    h20_d = din("h20", [ND, 128, NB])              # 2*h0 transposed
    w20_d = din("w20", [128, 4 * NB], F32)         # 2*c0  [(r, b) cols]
    fct_d = din("fct", [ND, 128, VOCAB])           # (fc/2).T [dec, vocab]
    out_d = nc.dram_tensor("out", [S * NB, VOCAB], F32, kind="ExternalOutput").ap()

    with tile.TileContext(nc) as tc:
        from contextlib import ExitStack

        with ExitStack() as glob_ctx:
            gp = glob_ctx.enter_context(tc.tile_pool(name="glob", bufs=1))
            # persistent state / loop constants
            encp = [gp.tile([128, PB], BF16, name=f"encp{i}", tag=f"encp{i}")
                    for i in range(NA)]
            argt = [gp.tile([128, PB], BF16, name=f"arg{i}", tag=f"arg{i}")
                    for i in range(NA)]
            attt = [gp.tile([128, PB], BF16, name=f"att{i}", tag=f"att{i}")
                    for i in range(NA)]
            ifp = [gp.tile([128, D4], BF16, name=f"ifp{i}", tag=f"ifp{i}")
                   for i in range(2 * NB)]
            wdect = [gp.tile([128, ATT], BF16, name=f"wdect{k}",
                             tag=f"wdect{k}") for k in range(ND)]
            vt = [gp.tile([128, 1], BF16, name=f"vt{i}", tag=f"vt{i}")
                  for i in range(NA)]
            encb = [gp.tile([128, 1], F32, name=f"encb{i}", tag=f"encb{i}")
                    for i in range(NA)]
            wdecb = [gp.tile([128, 1], F32, name=f"wdecb{i}", tag=f"wdecb{i}")
                     for i in range(NA)]
            ept = gp.tile([128, S * NMT * NB], BF16, name="ept")
            Hbig = [gp.tile([128, (S + 1) * NB], BF16, name=f"Hb{k}",
                            tag=f"Hb{k}") for k in range(ND)]
            W2 = gp.tile([128, 4 * NB], F32, name="W2")
            hp_sb = [gp.tile([128, NB], F32, name=f"hp{i}", tag=f"hp{i}")
                     for i in range(NA)]
            et0 = gp.tile([128, NB], F32, name="et0")
            et1 = gp.tile([128, NB], F32, name="et1")
            rsum = gp.tile([1, NB], F32, name="rsum")
            wt0 = gp.tile([128, NB], BF16, name="wt0")
            wt1 = gp.tile([128, NB], BF16, name="wt1")
            ctxsb = gp.tile([128, 128], BF16, name="ctxsb")
            gsb = [gp.tile([128, 4 * NB], F32, name=f"gsb{g}", tag=f"gsb{g}")
                   for g in range(4)]
            tact = [gp.tile([128, 4 * NB], F32, name=f"tact{g}",
                            tag=f"tact{g}") for g in range(4)]
            ucell = gp.tile([128, 4 * NB], F32, name="ucell")
            vcell = gp.tile([128, 4 * NB], F32, name="vcell")
            thc = gp.tile([128, 4 * NB], F32, name="thc")
            ones_col = gp.tile([128, 1], F32, name="ones_col")
            ones_row = gp.tile([1, 128], F32, name="ones_row")

            for k in range(ND):
                nc.sync.dma_start(out=wdect[k], in_=wdect_d[k])
            for i in range(NA):
                nc.sync.dma_start(out=vt[i], in_=vt_d[i])
                nc.sync.dma_start(out=encb[i], in_=encb_d[i])
                nc.sync.dma_start(out=wdecb[i], in_=wdecb_d[i])
            nc.sync.dma_start(out=ept, in_=ept_d)
            for k in range(ND):
                nc.sync.dma_start(out=Hbig[k][:, 0:NB], in_=h20_d[k])
            nc.sync.dma_start(out=W2, in_=w20_d)
            for i in range(2 * NB):
                nc.sync.dma_start(out=ifp[i], in_=ifp_d[i])
            nc.vector.memset(ones_col, 1.0)
            nc.vector.memset(ones_row, 1.0)

            # ---------------- pre-loop: enc_proj ----------------
            with tc.tile_pool(name="pre", bufs=1) as prep, \
                 tc.tile_pool(name="pspre", bufs=4, space="PSUM") as pspre:
                ift = [prep.tile([128, PB], BF16, name=f"ift{k}",
                                 tag=f"ift{k}") for k in range(NE)]
                wenct = [prep.tile([128, ATT], BF16, name=f"wen{k}",
                                   tag=f"wen{k}") for k in range(NE)]
                for k in range(NE):
                    nc.sync.dma_start(out=ift[k], in_=ift_d[k])
                    nc.sync.dma_start(out=wenct[k], in_=wenct_d[k])
                for i in range(NA):
                    for c in range(4):
                        ps = pspre.tile([128, EC], F32, name="eps", tag="mm")
                        for k in range(NE):
                            nc.tensor.matmul(
                                ps, wenct[k][:, i * 128:(i + 1) * 128],
                                ift[k][:, c * EC:(c + 1) * EC],
                                start=(k == 0), stop=(k == NE - 1))
                        nc.vector.tensor_scalar_add(
                            encp[i][:, c * EC:(c + 1) * EC], ps, encb[i])

            # ---------------- recurrence ----------------
            with tc.tile_pool(name="main", bufs=1) as mainp, \
                 tc.tile_pool(name="psl", bufs=1, space="PSUM") as psl:
                w2 = [mainp.tile([128, D4], BF16, name=f"w2_{k}",
                                 tag=f"w2_{k}") for k in range(NK2)]
                for k in range(NK2):
                    nc.sync.dma_start(out=w2[k], in_=w2ct_d[k])

                for t in range(S):
                    hof = t * NB
                    g_ps = psl.tile([128, 128], F32, name="g_ps", tag="g",
                                    bufs=2)
                    sm = psl.tile([128, 64], F32, name="sm", tag="sm", bufs=2)
                    ctx_ps = psl.tile([128, 128], F32, name="ctx", tag="ctx",
                                      bufs=2)
                    whh_ps = psl.tile([128, 128], F32, name="whh", tag="whh",
                                      bufs=2)
                    # hprojT = (wdec/2) @ H2  -> [a, b]
                    for i in range(NA):
                        for k in range(ND):
                            nc.tensor.matmul(
                                sm[:, i * 8:(i + 1) * 8],
                                wdect[k][:, i * 128:(i + 1) * 128],
                                Hbig[k][:, hof:hof + NB],
                                start=(k == 0), stop=(k == ND - 1),
                                skip_group_check=True)
                    for i in range(NA):
                        nc.vector.tensor_scalar_add(
                            hp_sb[i], sm[:, i * 8:(i + 1) * 8], wdecb[i])
                    # W_hh part of gates, issued early to keep PE busy.
                    # Own PSUM tile: a start=True matmul clears has_written
                    # for its whole bank, so this group cannot stay open
                    # across the attention-phase matmuls.
                    for mt in range(NMT):
                        for k in range(NE, NK2):
                            nc.tensor.matmul(
                                whh_ps[:, mt * 8:(mt + 1) * 8],
                                w2[k][:, mt * 128:(mt + 1) * 128],
                                Hbig[k - NE][:, hof:hof + NB],
                                start=(k == NE), stop=(k == NK2 - 1),
                                skip_group_check=True)
                    # gsb = whh + embproj early (off the critical path)
                    for g in range(4):
                        nc.vector.tensor_add(
                            gsb[g], whh_ps[:, g * 32:(g + 1) * 32],
                            ept[:, (t * NMT + g * 4) * NB:
                                (t * NMT + (g + 1) * 4) * NB])
                    # att = tanh(encp + hproj)
                    for i in range(NA):
                        for b in range(NB):
                            nc.vector.tensor_scalar_add(
                                argt[i][:, b * P:(b + 1) * P],
                                encp[i][:, b * P:(b + 1) * P],
                                hp_sb[i][:, b:b + 1])
                    for i in range(NA):
                        nc.scalar.activation(attt[i], argt[i], AF.Tanh)
                    # eT[q, b] = V . att
                    for b in range(NB):
                        lo = b * P
                        for i in range(NA):
                            nc.tensor.matmul(
                                sm[:, 32 + b:33 + b],
                                attt[i][:, lo:lo + 128], vt[i],
                                start=(i == 0), stop=(i == NA - 1),
                                skip_group_check=True)
                        for i in range(NA):
                            nc.tensor.matmul(
                                sm[0:J1, 40 + b:41 + b],
                                attt[i][:, lo + 128:lo + P], vt[i],
                                start=(i == 0), stop=(i == NA - 1),
                                skip_group_check=True)
                    # softmax over q (no max-subtraction; |e| <= sum|V| ~ 11)
                    nc.scalar.activation(et0, sm[:, 32:40], AF.Exp)
                    nc.scalar.activation(et1[0:J1], sm[0:J1, 40:48], AF.Exp)
                    nc.tensor.matmul(sm[0:1, 48:56], ones_col, et0,
                                     start=True, stop=False,
                                     skip_group_check=True)
                    nc.tensor.matmul(sm[0:1, 48:56], ones_col[0:J1],
                                     et1[0:J1], start=False, stop=True,
                                     skip_group_check=True)
                    nc.vector.reciprocal(rsum, sm[0:1, 48:56])
                    nc.tensor.matmul(sm[:, 56:64], ones_row, rsum,
                                     start=True, stop=True,
                                     skip_group_check=True)
                    nc.vector.tensor_mul(wt0, et0, sm[:, 56:64])
                    nc.vector.tensor_mul(wt1[0:J1], et1[0:J1],
                                         sm[0:J1, 56:64])
                    # stage 1: ctxT[e, b] = IF.T @ w
                    for et in range(NE):
                        eo = et * 128
                        for b in range(NB):
                            co = et * 8 + b
                            nc.tensor.matmul(
                                ctx_ps[:, co:co + 1],
                                ifp[2 * b][:, eo:eo + 128], wt0[:, b:b + 1],
                                start=True, stop=False, skip_group_check=True)
                            nc.tensor.matmul(
                                ctx_ps[:, co:co + 1],
                                ifp[2 * b + 1][0:J1, eo:eo + 128],
                                wt1[0:J1, b:b + 1],
                                start=False, stop=True, skip_group_check=True)
                    nc.vector.tensor_copy(out=ctxsb, in_=ctx_ps)
                    # stage 2: gates += Wc @ ctx
                    for mt in range(NMT):
                        for k in range(NE):
                            nc.tensor.matmul(
                                g_ps[:, mt * 8:(mt + 1) * 8],
                                w2[k][:, mt * 128:(mt + 1) * 128],
                                ctxsb[:, k * 8:(k + 1) * 8],
                                start=(k == 0), stop=(k == NE - 1),
                                skip_group_check=True)
                    # cell (all-tanh): sigma(x) = (1+tanh(x/2))/2
                    for g in range(4):
                        nc.vector.tensor_add(
                            gsb[g], gsb[g], g_ps[:, g * 32:(g + 1) * 32])
                    nc.scalar.activation(tact[0], gsb[0], AF.Tanh, scale=0.5)
                    nc.scalar.activation(tact[1], gsb[1], AF.Tanh, scale=0.5)
                    nc.scalar.activation(tact[2], gsb[2], AF.Tanh)
                    nc.scalar.activation(tact[3], gsb[3], AF.Tanh, scale=0.5)
                    nc.vector.scalar_tensor_tensor(
                        ucell, tact[1], 1.0, W2, ALU.add, ALU.mult)
                    nc.vector.scalar_tensor_tensor(
                        vcell, tact[0], 1.0, tact[2], ALU.add, ALU.mult)
                    nc.vector.scalar_tensor_tensor(
                        W2, ucell, 0.5, vcell, ALU.mult, ALU.add)
                    nc.scalar.activation(thc, W2, AF.Tanh, scale=0.5)
                    for r in range(ND):
                        nc.vector.scalar_tensor_tensor(
                            Hbig[r][:, hof + NB:hof + 2 * NB],
                            tact[3][:, r * 8:(r + 1) * 8], 1.0,
                            thc[:, r * 8:(r + 1) * 8], ALU.add, ALU.mult)

            # ---------------- tail: logits ----------------
            with tc.tile_pool(name="fc", bufs=2) as fcp, \
                 tc.tile_pool(name="pst", bufs=4, space="PSUM") as pst, \
                 tc.tile_pool(name="lgp", bufs=4) as lgp:
                for c in range(NVC):
                    fcs = [fcp.tile([128, VC], BF16, name=f"fcs{k}",
                                    tag=f"fcs{k}") for k in range(ND)]
                    for k in range(ND):
                        nc.sync.dma_start(
                            out=fcs[k],
                            in_=fct_d[k][:, c * VC:(c + 1) * VC])
                    for mi, (m0, msz) in enumerate(
                            ((0, 128), (128, S * NB - 128))):
                        ps = pst.tile([128, VC], F32, name="lps", tag="l")
                        for k in range(ND):
                            nc.tensor.matmul(
                                ps[:msz],
                                Hbig[k][:, NB + m0:NB + m0 + msz],
                                fcs[k],
                                start=(k == 0), stop=(k == ND - 1))
                        lg = lgp.tile([128, VC], F32, name="lg", tag="lg")
                        if c % 2 == 0:
                            nc.scalar.copy(out=lg[:msz], in_=ps[:msz])
                        else:
                            nc.vector.tensor_copy(out=lg[:msz],
                                                  in_=ps[:msz])
                        nc.sync.dma_start(
                            out=out_d[m0:m0 + msz, c * VC:(c + 1) * VC],
                            in_=lg[:msz])
    nc.compile()
    _CACHE["nc"] = nc
    return nc


def _prep_core_inputs(image_feat, embproj, h0, c0, wenct, w2ct, wdect,
                      vt, fct, encb, wdecb, core):
    bs = slice(core * NB, (core + 1) * NB)
    ifc = image_feat[bs]                                # [8, 196, 2048] f32
    ift = np.ascontiguousarray(
        ifc.reshape(NB * P, ENC).T).astype(BF).reshape(NE, 128, PB)
    ifp = np.zeros((2 * NB, 128, ENC), BF)
    for b in range(NB):
        ifp[2 * b] = ifc[b, 0:128, :].astype(BF)
        ifp[2 * b + 1, 0:J1] = ifc[b, 128:P, :].astype(BF)
    ep = embproj[bs]                                    # [8, 20, 2048]
    ept = np.ascontiguousarray(
        ep.transpose(2, 1, 0)                           # [2048, 20, 8]
        .reshape(NMT, 128, S, NB)                       # [mt, r, t, b]
        .transpose(1, 2, 0, 3)                          # [r, t, mt, b]
        .reshape(128, S * NMT * NB)).astype(BF)
    h20 = np.ascontiguousarray(
        (2.0 * h0[bs]).T).reshape(ND, 128, NB).astype(BF)
    w20 = np.ascontiguousarray(
        (2.0 * c0[bs]).T.reshape(ND, 128, NB)
        .transpose(1, 0, 2).reshape(128, ND * NB)).astype(np.float32)
    return dict(ift=ift, ifp=ifp, wenct=wenct, w2ct=w2ct, wdect=wdect,
                vt=vt, ept=ept, h20=h20, w20=w20, fct=fct, encb=encb,
                wdecb=wdecb)


def kernel(image_feat, captions_ids, wenc_w, wenc_b, wdec_w, wdec_b,
           V_w, V_b, embed_w, h0_w, h0_b, c0_w, c0_b,
           W_ih, b_ih, W_hh, b_hh, fc_w, fc_b):
    image_feat = np.asarray(image_feat, np.float32)
    ids = np.asarray(captions_ids).astype(np.int64)

    # host-side glue (cheap, not on the device critical path)
    emb_seq = np.asarray(embed_w, np.float32)[ids]            # [B, S, EMB]
    We = np.asarray(W_ih, np.float32)[:, ENC:]                # [D4, EMB]
    Wc = np.asarray(W_ih, np.float32)[:, :ENC]                # [D4, ENC]
    embproj = emb_seq @ We.T + (np.asarray(b_ih) + np.asarray(b_hh))
    avg = image_feat.mean(axis=1)
    h0 = np.maximum(avg @ np.asarray(h0_w, np.float32).T + h0_b, 0.0)
    c0 = np.maximum(avg @ np.asarray(c0_w, np.float32).T + c0_b, 0.0)

    wenct = np.ascontiguousarray(
        np.asarray(wenc_w, np.float32).T).astype(BF).reshape(NE, 128, ATT)
    w2ct = np.concatenate([
        np.ascontiguousarray(Wc.T),
        np.ascontiguousarray(0.5 * np.asarray(W_hh, np.float32).T)],
        axis=0).astype(BF).reshape(NK2, 128, D4)
    wdect = np.ascontiguousarray(
        0.5 * np.asarray(wdec_w, np.float32).T).astype(BF).reshape(
        ND, 128, ATT)
    vtt = np.ascontiguousarray(
        np.asarray(V_w, np.float32)[0]).astype(BF).reshape(NA, 128, 1)
    fct = np.ascontiguousarray(
        0.5 * np.asarray(fc_w, np.float32).T).astype(BF).reshape(
        ND, 128, VOCAB)
    encb = np.asarray(wenc_b, np.float32).reshape(NA, 128, 1)
    wdecb = np.asarray(wdec_b, np.float32).reshape(NA, 128, 1)

    nc = _build_nc()
    in_maps = [
        _prep_core_inputs(image_feat, embproj, h0, c0, wenct, w2ct, wdect,
                          vtt, fct, encb, wdecb, c)
        for c in range(NCORES)
    ]
    res = run_bass_kernel_spmd(nc, in_maps, core_ids=list(range(NCORES)),
                               trace=TRACE)
    if TRACE:
        _CACHE["last_results"] = res

    preds = np.empty((B, S, VOCAB), np.float32)
    for c in range(NCORES):
        lg = res.results[c]["out"].reshape(S, NB, VOCAB)
        preds[c * NB:(c + 1) * NB] = lg.transpose(1, 0, 2)
    preds += np.asarray(fc_b, np.float32)
    return preds


if __name__ == "__main__":
    sys.path.insert(0, os.path.dirname(os.path.abspath(__file__)))
    import reference

    inputs = reference.setup_inputs()
    inputs = {k: np.asarray(v) for k, v in inputs.items()}
    expected = np.asarray(reference.reference(**inputs))
    actual = kernel(**inputs)
    err = np.abs(actual - expected)
    rel = np.linalg.norm(actual - expected) / np.linalg.norm(expected)
    print("max abs err:", err.max(), "rel:", rel)
